# revision 43
# baseline (speedup 1.0000x reference)
"""Trainium2 Bass kernel for pairwise-MLP GNN message passing.

Computation (per batch b, position l):
    x[i,j] = concat(states[l,i], states[l,j])           # [N,N,2D]
    out    = sigmoid(MLP(x))                            # [N,N,8], MLP: 32->64->64->8

Factorization used on device: the first linear layer splits into
A = states @ W1[:D] + b1 and B = states @ W1[D:], so
h1[i,j] = relu(A[i] + B[j]) — the N^2 expansion happens as a cheap
broadcast add on the vector engine instead of an N^2-row matmul.

Sharding: data-parallel over batch, core c <- batch c (8 cores, B=8).

Device design (per core, L=64 l-blocks; 2 l-blocks = 1 "sb"):
  - features on partitions: partitions 0:64 = even l-block of the sb,
    64:128 = odd l-block (host-side shifted copy of states^T feeding
    block-diagonal-packed matmuls); pair columns col = 32*i + j.
  - S1 (broadcast add + relu) runs on VectorE (tensor_tensor at 2x mode
    via the duplicated-A operand + tensor_scalar relu at 4x) in 2-sb
    chunks at the head (fast pipeline fill), 4-sb chunks mid-stream
    (amortize op startup), and per-sb ops at the tail (short final
    L2->evict->L3->sigmoid chain). A2dup evictions run on ScalarE and
    B2 on the (otherwise idle) DVE during L1 so S1 starts early.
  - L2: ONE K=128/M=128 block-diagonal matmul per 512 cols (measured:
    matmul cost is ~216ns per 512-col stream regardless of K/M in
    {64,128}) — 2 matmuls/sb, half the PE time of quadrant packing.
  - h2 eviction [128, 1024] per sb: ScalarE activation(Relu, bias) for
    most sbs, DVE tensor_scalar for the sbs in H2EV_DVE (engine load
    balance; DVE's S1 stream finishes ~2/3 through the kernel).
  - L3: K=128/M=32 matmuls with W3 packed at partition offsets 0:8
    (even l-block) / 16:24 (odd); tile_position col offsets 0/32/64/96
    place the four (sb-in-pair, col-chunk) units of a pair in one
    [128, 512] psum bank -> ONE [128, 512] sigmoid per pair (4x fewer
    ScalarE sigmoid columns than a [128,1024]-per-pair layout).
  - Output: one [128, 512] bf16 DMA per pair (dispatch alternates
    sync/gpsimd queues); host decodes the 32 valid rows.
"""

import os
import sys

import numpy as np

for _p in ("/opt/trn_rl_repo", "/root/.axon_site/_ro/trn_rl_repo"):
    if os.path.isdir(_p) and _p not in sys.path:
        sys.path.insert(0, _p)

from concourse import bacc, mybir, tile
from concourse.bass_utils import run_bass_kernel_spmd

B, L, N, D = 8, 64, 32, 16
H = 64            # hidden width (h1 and h2)
F = 8             # out_dim
NCORES = 8
NSB = L // 2      # 32 superblocks per core
NPAIR = NSB // 2  # 16 pairs per core
COLS = N * N      # 1024 pair columns per l-block

# sbs whose h2 eviction runs on VectorE (rest on ScalarE); spread through
# the mid/late phase so the DVE's S1 stream stays ahead of L2.
H2EV_DVE = frozenset({10, 13, 16, 19, 22, 24, 26, 28, 31})

FP32 = mybir.dt.float32
BF16 = mybir.dt.bfloat16
NP_BF16 = mybir.dt.np(BF16)

_PROGRAM = None  # (nc, input_names)
LAST_RESULT = None  # BassKernelResults of the most recent kernel() call


def _build_program():
    nc = bacc.Bacc("TRN2", target_bir_lowering=False, debug=False)

    d_statesQ = nc.dram_tensor("statesQ", [32, 2048], BF16, kind="ExternalInput").ap()
    d_Wl1 = nc.dram_tensor("Wl1", [32, 256], BF16, kind="ExternalInput").ap()
    d_W2blk = nc.dram_tensor("W2blk", [128, 128], BF16, kind="ExternalInput").ap()
    d_W3v = nc.dram_tensor("W3v", [128, 32], BF16, kind="ExternalInput").ap()
    d_biases = nc.dram_tensor("biases", [128, 3], FP32, kind="ExternalInput").ap()
    d_out = nc.dram_tensor("out", [NPAIR, 128, 512], BF16, kind="ExternalOutput").ap()

    add = mybir.AluOpType.add
    max_ = mybir.AluOpType.max
    AF = mybir.ActivationFunctionType

    with tile.TileContext(nc) as tc:
        with tc.tile_pool(name="const", bufs=1) as const_pool:
            statesQ = const_pool.tile([32, 2048], BF16, name="statesQ_t")[:]
            Wl1 = const_pool.tile([32, 256], BF16, name="Wl1_t")[:]
            W2blk = const_pool.tile([128, 128], BF16, name="W2blk_t")[:]
            W3v = const_pool.tile([128, 32], BF16, name="W3v_t")[:]
            biases = const_pool.tile([128, 3], FP32, name="biases_t")[:]
            bias1 = biases[:, 0:1]
            bias2 = biases[:, 1:2]
            bias3 = biases[:, 2:3]
            A2dup = const_pool.tile([128, 2 * COLS], BF16, name="A2dup_t")[:]
            B2s = const_pool.tile([128, COLS], BF16, name="B2s_t")[:]

            # Parallel dispatch across queues: serial dma_start dispatch costs
            # ~700ns each and would otherwise delay the L1 matmuls.
            nc.sync.dma_start(out=statesQ[0:16], in_=d_statesQ[0:16])
            nc.gpsimd.dma_start(out=statesQ[16:32], in_=d_statesQ[16:32])
            nc.scalar.dma_start(out=Wl1, in_=d_Wl1)
            nc.scalar.dma_start(out=biases, in_=d_biases)
            nc.sync.dma_start(out=W2blk, in_=d_W2blk)
            nc.gpsimd.dma_start(out=W3v, in_=d_W3v)


            # Warm the ACT table with the sigmoid set up front: Identity/Relu
            # are filler functions present in every set, so this is the only
            # ACT_TABLE_LOAD the kernel pays. Reads the framework's memset
            # const tile, NOT an input, so the load doesn't wait on any DMA.
            sigwarm = const_pool.tile([128, 1], FP32, name="sigwarm_t")[:]
            nc.scalar.activation(sigwarm, nc.const_aps.aps[(FP32, 0.0)], AF.Sigmoid)

            # ---- Layer 1: A2/B2 = per-agent halves of the first linear layer.
            # A2[p, 32*sb + i]: p<64 -> even l-block (2sb), p>=64 -> odd (2sb+1)
            # — K=32 block-diagonal over the parities (statesQ rows 0:16 even,
            # 16:32 odd-shifted), so L1 is 4 matmuls. Column-chunk (sbh) outer
            # so the first 512 A2/B2 cols finish first and their evictions
            # (split below) unblock the DVE's S1 stream earlier.
            with tc.tile_pool(name="abps", bufs=1, space="PSUM") as ab_pool:
                A2ps = ab_pool.tile([128, COLS], FP32, tag="a2", name="A2ps_t")[:]
                B2ps = ab_pool.tile([128, COLS], FP32, tag="b2", name="B2ps_t")[:]
                rhs = statesQ.rearrange("p (s c) -> p s c", s=32)
                for sbh in (0, 1):
                    for w_lo, ps in ((0, A2ps), (128, B2ps)):
                        nc.tensor.matmul(
                            ps[:, 512 * sbh : 512 * sbh + 512],
                            Wl1[:, w_lo : w_lo + 128],
                            rhs[:, 16 * sbh : 16 * sbh + 16, 0:32],
                        )
                # Evict A2 twice (duplicated pairs so the later broadcast add
                # keeps an innermost unit stride), folding in b1; B2 plain.
                # All on ScalarE (the DVE starts S1 with no preliminaries),
                # small head chunks first: S1 chunk 0 only needs cols 0:256.
                dupview = A2dup.rearrange("p (c two) -> p two c", two=2)
                for c0, c1 in ((0, 256), (256, 512), (512, 1024)):
                    nc.scalar.activation(
                        dupview[:, 0, c0:c1], A2ps[:, c0:c1], AF.Identity, bias=bias1
                    )
                    nc.scalar.activation(
                        dupview[:, 1, c0:c1], A2ps[:, c0:c1], AF.Identity, bias=bias1
                    )
                    # B2 on the DVE: it idles here anyway until A2dup lands.
                    nc.vector.tensor_scalar_add(B2s[:, c0:c1], B2ps[:, c0:c1], 0.0)

            with (
                tc.tile_pool(name="h1pre", bufs=2) as h1pre_pool,
                tc.tile_pool(name="h1", bufs=4) as h1_pool,
                tc.tile_pool(name="h2", bufs=6) as h2_pool,
                tc.tile_pool(name="sigp", bufs=3) as sig_pool,
                tc.tile_pool(name="l2ps", bufs=3, space="PSUM") as l2_pool,
                tc.tile_pool(name="l3ps", bufs=2, space="PSUM") as l3_pool,
            ):
                h2_tiles = {}     # sb -> h2 AP
                psum3_tiles = {}  # pair -> psum3 AP
                h1_bufs = {}      # quad -> h1 AP
                h1pre_bufs = {}

                def emit_s1_tt(sb2, nsb=2, eng=None):
                    # S1 chunks cover nsb sbs each: small chunks early (so the
                    # pipeline fills fast), 4-sb chunks mid-stream (amortizes
                    # the ~140ns/op DVE startup cost).
                    quad, h4 = divmod(sb2, 2)
                    if h4 == 0:
                        h1pre_bufs[quad] = h1pre_pool.tile(
                            [128, 4 * COLS], BF16, tag="h1pre", name="h1pre_t"
                        )[:]
                        h1_bufs[quad] = h1_pool.tile(
                            [128, 4 * COLS], BF16, tag="h1", name="h1_t"
                        )[:]
                    h1pre = h1pre_bufs[quad]
                    a_in = (
                        A2dup[:, 128 * sb2 : 128 * sb2 + 64 * nsb]
                        .rearrange("p (s i two) -> p s i two", s=nsb, two=2)
                        .unsqueeze(3)
                        .broadcast_to([128, nsb, 32, 16, 2])
                    )
                    b_in = (
                        B2s[:, 64 * sb2 : 64 * sb2 + 32 * nsb]
                        .rearrange("p (s jh jl) -> p s jh jl", s=nsb, jl=2)
                        .unsqueeze(2)
                        .broadcast_to([128, nsb, 32, 16, 2])
                    )
                    h1pre_v = h1pre[
                        :, 2 * COLS * h4 : 2 * COLS * h4 + nsb * COLS
                    ].rearrange("p (s i jh jl) -> p s i jh jl", s=nsb, i=32, jl=2)
                    (eng or nc.vector).tensor_add(h1pre_v, a_in, b_in)

                def emit_s1_relu(sb2, nsb=2):
                    quad, h4 = divmod(sb2, 2)
                    cs = slice(2 * COLS * h4, 2 * COLS * h4 + nsb * COLS)
                    nc.vector.tensor_scalar_max(
                        h1_bufs[quad][:, cs], h1pre_bufs[quad][:, cs], 0.0
                    )

                def emit_s1_single(sb):
                    # 1-sb S1 (used for the last two sbs): halves the latency
                    # from "h1 ready" to the tail L2->evict->L3->sigmoid chain.
                    quad, k4 = divmod(sb, 4)
                    h1pre = h1pre_bufs[quad]
                    a_in = (
                        A2dup[:, 64 * sb : 64 * sb + 64]
                        .rearrange("p (i two) -> p i two", two=2)
                        .unsqueeze(2)
                        .broadcast_to([128, 32, 16, 2])
                    )
                    b_in = (
                        B2s[:, 32 * sb : 32 * sb + 32]
                        .rearrange("p (jh jl) -> p jh jl", jl=2)
                        .unsqueeze(1)
                        .broadcast_to([128, 32, 16, 2])
                    )
                    cs = slice(COLS * k4, COLS * k4 + COLS)
                    h1pre_v = h1pre[:, cs].rearrange(
                        "p (i jh jl) -> p i jh jl", i=32, jl=2
                    )
                    nc.vector.tensor_add(h1pre_v, a_in, b_in)
                    nc.vector.tensor_scalar_max(h1_bufs[quad][:, cs], h1pre[:, cs], 0.0)

                def emit_l2_s2(sb, hk):
                    # K=128 M=128 block-diagonal W2: one matmul per 512 cols.
                    psum2 = l2_pool.tile([128, COLS], FP32, tag="l2", name="psum2_t")[:]
                    nc.tensor.matmul(psum2[:, 0:512], W2blk, hk[:, 0:512])
                    nc.tensor.matmul(psum2[:, 512:1024], W2blk, hk[:, 512:1024])
                    h2 = h2_pool.tile([128, COLS], BF16, tag="h2", name="h2_t")[:]
                    if sb in H2EV_DVE:
                        nc.vector.tensor_scalar(h2, psum2, bias2, 0.0, add, max_)
                    else:
                        nc.scalar.activation(h2, psum2, AF.Relu, bias=bias2)
                    h2_tiles[sb] = h2

                def emit_l3(sb):
                    # K=128 M=32: W3v packs W3 for both l-block parities into
                    # rows 0:8 / 16:24 of a 32-wide M block; tile_position col
                    # offsets place the pair's four (sb,chunk) units in one
                    # [128, 512] psum bank.
                    pair, k = divmod(sb, 2)
                    if k == 0:
                        psum3_tiles[pair] = l3_pool.tile(
                            [128, 512], FP32, tag="l3", name="psum3_t"
                        )[:]
                    psum3 = psum3_tiles[pair]
                    h2 = h2_tiles.pop(sb)
                    for ch in (0, 1):
                        u = 2 * k + ch
                        nc.tensor.matmul(
                            psum3[32 * u : 32 * u + 32, :],
                            W3v,
                            h2[:, 512 * ch : 512 * ch + 512],
                            tile_position=(0, 32 * u),
                        )

                def emit_sigmoid_dma(pair):
                    sig = sig_pool.tile([128, 512], BF16, tag="sig", name="sig_t")[:]
                    psum3 = psum3_tiles.pop(pair)
                    nc.scalar.activation(sig, psum3, AF.Sigmoid, bias=bias3)
                    if pair == NPAIR - 1:
                        # Split the last transfer so the drain tail halves.
                        nc.sync.dma_start(out=d_out[pair, 0:64], in_=sig[0:64])
                        nc.gpsimd.dma_start(out=d_out[pair, 64:128], in_=sig[64:128])
                    else:
                        eng = nc.sync if pair % 2 == 0 else nc.gpsimd
                        eng.dma_start(out=d_out[pair], in_=sig)

                # Software pipeline, deliberately deep so no strict-FIFO queue
                # ever stalls at its head: S1 runs two quads (8 sbs) ahead of
                # L2, in per-2sb chunks; L3 lags L2 by two sbs (its h2 is long
                # since evicted); the sigmoid lags its pair's last L3 too.
                # S1 emission plan: 2-sb chunks up front (fast pipeline fill),
                # 4-sb (full-quad) chunks mid-stream, per-sb ops at the tail.
                s1_plan = {2: 4, 6: 6, 10: 8, 14: 10, 18: 12}
                for sb2 in range(4):
                    emit_s1_tt(sb2)
                    emit_s1_relu(sb2)
                for sb in range(NSB):
                    quad, k4 = divmod(sb, 4)
                    if sb in s1_plan:
                        emit_s1_tt(s1_plan[sb], nsb=4)
                        emit_s1_relu(s1_plan[sb], nsb=4)
                    elif sb == 22:
                        emit_s1_tt(14)
                        emit_s1_relu(14)
                    elif sb == 24:
                        emit_s1_single(30)
                        emit_s1_single(31)
                    emit_l2_s2(sb, h1_bufs[quad][:, COLS * k4 : COLS * k4 + COLS])
                    if sb >= 2:
                        emit_l3(sb - 2)
                    if sb % 2 == 1 and sb >= 3:  # pair (sb-3)//2 L3'd at sb-1
                        emit_sigmoid_dma((sb - 3) // 2)
                emit_l3(NSB - 2)
                emit_l3(NSB - 1)
                emit_sigmoid_dma(NPAIR - 1)

    nc.compile()
    input_names = ["statesQ", "Wl1", "W2blk", "W3v", "biases"]
    return nc, input_names


def get_program():
    global _PROGRAM
    if _PROGRAM is None:
        _PROGRAM = _build_program()
    return _PROGRAM


def make_inputs(states, W1, b1, W2, b2, W3, b3):
    """Host-side prep: per-core statesQ + shared packed weights/biases."""
    states = np.asarray(states, np.float32)
    W1 = np.asarray(W1, np.float32)
    W2 = np.asarray(W2, np.float32)
    W3 = np.asarray(W3, np.float32)
    b1 = np.asarray(b1, np.float32)
    b2 = np.asarray(b2, np.float32)
    b3 = np.asarray(b3, np.float32)

    # Wl1 [32, 256]: cols 0:128 = K=32 parity-blockdiag lhsT for A2
    # (rows 0:16 -> outs 0:64 even, rows 16:32 -> outs 64:128 odd),
    # cols 128:256 = same structure for B2 (second half of W1).
    Wl1 = np.zeros((32, 256), NP_BF16)
    Wl1[0:16, 0:64] = W1[:D].astype(NP_BF16)
    Wl1[16:32, 64:128] = W1[:D].astype(NP_BF16)
    Wl1[0:16, 128:192] = W1[D:].astype(NP_BF16)
    Wl1[16:32, 192:256] = W1[D:].astype(NP_BF16)

    W2blk = np.zeros((128, 128), NP_BF16)
    W2blk[0:64, 0:64] = W2.astype(NP_BF16)
    W2blk[64:128, 64:128] = W2.astype(NP_BF16)

    # W3v: K=128 M=32 weight for L3 — even-parity h2 (parts 0:64) feeds out
    # rows 0:8, odd-parity (64:128) feeds rows 16:24.
    W3v = np.zeros((128, 32), NP_BF16)
    W3v[0:64, 0:8] = W3.astype(NP_BF16)
    W3v[64:128, 16:24] = W3.astype(NP_BF16)

    biases = np.zeros((128, 3), np.float32)
    biases[:, 0] = np.tile(b1, 2)
    biases[:, 1] = np.tile(b2, 2)
    b3pat = np.zeros(32, np.float32)
    b3pat[0:8] = b3
    b3pat[16:24] = b3
    biases[:, 2] = np.tile(b3pat, 4)

    shared = {"Wl1": Wl1, "W2blk": W2blk, "W3v": W3v, "biases": biases}

    in_maps = []
    for c in range(NCORES):
        # statesT[d, 32*l + i] = states[c, l, i, d]
        statesT = states[c].reshape(L * N, D).T.astype(NP_BF16)
        sQ = np.zeros((32, 2048), NP_BF16)
        sQ[0:16] = statesT
        sQ[16:32, : 2048 - 32] = statesT[:, 32:]
        in_maps.append({"statesQ": sQ, **shared})
    return in_maps


def decode_output(raw):
    """Invert the device output layout -> [L, N, N, F] for one core.

    raw: [NPAIR, 128, 512] bf16. Partitions: u = 2*s + ch in blocks of 32
    (s = sb within pair, ch = column chunk), rows 0:8 = even l-block
    features, 16:24 = odd. col c: pair-col = 512*ch + c = 32*i + j.
    l = 4*pair + 2*s + parity.
    """
    raw = raw.astype(np.float32)
    ov = raw.reshape(NPAIR, 2, 2, 32, 512)       # [pair, s, ch, row, c]
    ev = ov[:, :, :, 0:8, :]                     # even parity
    od = ov[:, :, :, 16:24, :]                   # odd parity
    st = np.stack([ev, od], axis=3)              # [pair, s, ch, par, f, c]
    st = st.transpose(0, 1, 3, 2, 5, 4)          # [pair, s, par, ch, c, f]
    return np.ascontiguousarray(st.reshape(L, N * N, F)).reshape(L, N, N, F)


def _ensure_ntff_hook():
    """Best-effort shim for the missing antenv.axon_hooks module so
    run_bass_kernel_spmd(trace=True) can capture NTFF profiles under axon."""
    import types

    try:
        from antenv.axon_hooks import get_axon_ntff_profile_hook  # noqa: F401
        return
    except ImportError:
        pass
    try:
        if "/root/.axon_site" not in sys.path:
            sys.path.insert(0, "/root/.axon_site")
        from trn_agent_boot.trn_boot import _ntff_profile_via_ctypes

        hook = _ntff_profile_via_ctypes("/opt/axon/libaxon_pjrt.so")
        import antenv

        mod = types.ModuleType("antenv.axon_hooks")
        mod._hook = hook
        mod.set_axon_ntff_profile_hook = lambda h: setattr(mod, "_hook", h)
        mod.get_axon_ntff_profile_hook = lambda: mod._hook
        sys.modules["antenv.axon_hooks"] = mod
        antenv.axon_hooks = mod
    except Exception as e:  # tracing is optional; never break the run
        print(f"ntff hook shim failed: {e}", file=sys.stderr)


def kernel(states, W1, b1, W2, b2, W3, b3):
    global LAST_RESULT
    nc, _ = get_program()
    if os.environ.get("KERNEL_TRACE"):
        _ensure_ntff_hook()
    in_maps = make_inputs(states, W1, b1, W2, b2, W3, b3)
    res = run_bass_kernel_spmd(
        nc,
        in_maps,
        core_ids=list(range(NCORES)),
        trace=bool(os.environ.get("KERNEL_TRACE")),
    )
    LAST_RESULT = res
    out = np.empty((B, L, N, N, F), np.float32)
    for c in range(NCORES):
        out[c] = decode_output(res.results[c]["out"])
    return out


# revision 44
# speedup vs baseline: 1.0007x; 1.0007x over previous
"""Trainium2 Bass kernel for pairwise-MLP GNN message passing.

Computation (per batch b, position l):
    x[i,j] = concat(states[l,i], states[l,j])           # [N,N,2D]
    out    = sigmoid(MLP(x))                            # [N,N,8], MLP: 32->64->64->8

Factorization used on device: the first linear layer splits into
A = states @ W1[:D] + b1 and B = states @ W1[D:], so
h1[i,j] = relu(A[i] + B[j]) — the N^2 expansion happens as a cheap
broadcast add on the vector engine instead of an N^2-row matmul.

Sharding: data-parallel over batch, core c <- batch c (8 cores, B=8).

Device design (per core, L=64 l-blocks; 2 l-blocks = 1 "sb"):
  - features on partitions: partitions 0:64 = even l-block of the sb,
    64:128 = odd l-block (host-side shifted copy of states^T feeding
    block-diagonal-packed matmuls); pair columns col = 32*i + j.
  - S1 (broadcast add + relu) runs on VectorE (tensor_tensor at 2x mode
    via the duplicated-A operand + tensor_scalar relu at 4x) in 2-sb
    chunks at the head (fast pipeline fill), 4-sb chunks mid-stream
    (amortize op startup), and per-sb ops at the tail (short final
    L2->evict->L3->sigmoid chain). A2dup evictions run on ScalarE and
    B2 on the (otherwise idle) DVE during L1 so S1 starts early.
  - L2: ONE K=128/M=128 block-diagonal matmul per 512 cols (measured:
    matmul cost is ~216ns per 512-col stream regardless of K/M in
    {64,128}) — 2 matmuls/sb, half the PE time of quadrant packing.
  - h2 eviction [128, 1024] per sb: ScalarE activation(Relu, bias) for
    most sbs, DVE tensor_scalar for the sbs in H2EV_DVE (engine load
    balance; DVE's S1 stream finishes ~2/3 through the kernel).
  - L3: K=128/M=32 matmuls with W3 packed at partition offsets 0:8
    (even l-block) / 16:24 (odd); tile_position col offsets 0/32/64/96
    place the four (sb-in-pair, col-chunk) units of a pair in one
    [128, 512] psum bank -> ONE [128, 512] sigmoid per pair (4x fewer
    ScalarE sigmoid columns than a [128,1024]-per-pair layout).
  - Output: one [128, 512] bf16 DMA per pair (dispatch alternates
    sync/gpsimd queues); host decodes the 32 valid rows.
"""

import os
import sys

import numpy as np

for _p in ("/opt/trn_rl_repo", "/root/.axon_site/_ro/trn_rl_repo"):
    if os.path.isdir(_p) and _p not in sys.path:
        sys.path.insert(0, _p)

from concourse import bacc, mybir, tile
from concourse.bass_utils import run_bass_kernel_spmd

B, L, N, D = 8, 64, 32, 16
H = 64            # hidden width (h1 and h2)
F = 8             # out_dim
NCORES = 8
NSB = L // 2      # 32 superblocks per core
NPAIR = NSB // 2  # 16 pairs per core
COLS = N * N      # 1024 pair columns per l-block

# sbs whose h2 eviction runs on VectorE (rest on ScalarE); spread through
# the mid/late phase so the DVE's S1 stream stays ahead of L2.
H2EV_DVE = frozenset({13, 17, 21, 25, 28, 31})

FP32 = mybir.dt.float32
BF16 = mybir.dt.bfloat16
NP_BF16 = mybir.dt.np(BF16)

_PROGRAM = None  # (nc, input_names)
LAST_RESULT = None  # BassKernelResults of the most recent kernel() call


def _build_program():
    nc = bacc.Bacc("TRN2", target_bir_lowering=False, debug=False)

    d_statesQ = nc.dram_tensor("statesQ", [32, 2048], BF16, kind="ExternalInput").ap()
    d_Wl1 = nc.dram_tensor("Wl1", [32, 256], BF16, kind="ExternalInput").ap()
    d_W2blk = nc.dram_tensor("W2blk", [128, 128], BF16, kind="ExternalInput").ap()
    d_W3v = nc.dram_tensor("W3v", [128, 32], BF16, kind="ExternalInput").ap()
    d_biases = nc.dram_tensor("biases", [128, 3], FP32, kind="ExternalInput").ap()
    d_out = nc.dram_tensor("out", [NPAIR, 128, 512], BF16, kind="ExternalOutput").ap()

    add = mybir.AluOpType.add
    max_ = mybir.AluOpType.max
    AF = mybir.ActivationFunctionType

    with tile.TileContext(nc) as tc:
        with tc.tile_pool(name="const", bufs=1) as const_pool:
            statesQ = const_pool.tile([32, 2048], BF16, name="statesQ_t")[:]
            Wl1 = const_pool.tile([32, 256], BF16, name="Wl1_t")[:]
            W2blk = const_pool.tile([128, 128], BF16, name="W2blk_t")[:]
            W3v = const_pool.tile([128, 32], BF16, name="W3v_t")[:]
            biases = const_pool.tile([128, 3], FP32, name="biases_t")[:]
            bias1 = biases[:, 0:1]
            bias2 = biases[:, 1:2]
            bias3 = biases[:, 2:3]
            A2dup = const_pool.tile([128, 2 * COLS], BF16, name="A2dup_t")[:]
            B2s = const_pool.tile([128, COLS], BF16, name="B2s_t")[:]

            # Parallel dispatch across queues: serial dma_start dispatch costs
            # ~700ns each and would otherwise delay the L1 matmuls.
            nc.sync.dma_start(out=statesQ[0:16], in_=d_statesQ[0:16])
            nc.gpsimd.dma_start(out=statesQ[16:32], in_=d_statesQ[16:32])
            nc.scalar.dma_start(out=Wl1, in_=d_Wl1)
            nc.scalar.dma_start(out=biases, in_=d_biases)
            nc.sync.dma_start(out=W2blk, in_=d_W2blk)
            nc.gpsimd.dma_start(out=W3v, in_=d_W3v)


            # Warm the ACT table with the sigmoid set up front: Identity/Relu
            # are filler functions present in every set, so this is the only
            # ACT_TABLE_LOAD the kernel pays. Reads the framework's memset
            # const tile, NOT an input, so the load doesn't wait on any DMA.
            sigwarm = const_pool.tile([128, 1], FP32, name="sigwarm_t")[:]
            nc.scalar.activation(sigwarm, nc.const_aps.aps[(FP32, 0.0)], AF.Sigmoid)

            # ---- Layer 1: A2/B2 = per-agent halves of the first linear layer.
            # A2[p, 32*sb + i]: p<64 -> even l-block (2sb), p>=64 -> odd (2sb+1)
            # — K=32 block-diagonal over the parities (statesQ rows 0:16 even,
            # 16:32 odd-shifted), so L1 is 4 matmuls. Column-chunk (sbh) outer
            # so the first 512 A2/B2 cols finish first and their evictions
            # (split below) unblock the DVE's S1 stream earlier.
            with tc.tile_pool(name="abps", bufs=1, space="PSUM") as ab_pool:
                A2ps = ab_pool.tile([128, COLS], FP32, tag="a2", name="A2ps_t")[:]
                B2ps = ab_pool.tile([128, COLS], FP32, tag="b2", name="B2ps_t")[:]
                rhs = statesQ.rearrange("p (s c) -> p s c", s=32)
                for sbh in (0, 1):
                    for w_lo, ps in ((0, A2ps), (128, B2ps)):
                        nc.tensor.matmul(
                            ps[:, 512 * sbh : 512 * sbh + 512],
                            Wl1[:, w_lo : w_lo + 128],
                            rhs[:, 16 * sbh : 16 * sbh + 16, 0:32],
                        )
                # Evict A2 twice (duplicated pairs so the later broadcast add
                # keeps an innermost unit stride), folding in b1; B2 plain.
                # All on ScalarE (the DVE starts S1 with no preliminaries),
                # small head chunks first: S1 chunk 0 only needs cols 0:256.
                dupview = A2dup.rearrange("p (c two) -> p two c", two=2)
                for c0, c1 in ((0, 256), (256, 512), (512, 1024)):
                    nc.scalar.activation(
                        dupview[:, 0, c0:c1], A2ps[:, c0:c1], AF.Identity, bias=bias1
                    )
                    nc.scalar.activation(
                        dupview[:, 1, c0:c1], A2ps[:, c0:c1], AF.Identity, bias=bias1
                    )
                    # B2 on the DVE: it idles here anyway until A2dup lands.
                    nc.vector.tensor_scalar_add(B2s[:, c0:c1], B2ps[:, c0:c1], 0.0)

            with (
                tc.tile_pool(name="h1pre", bufs=2) as h1pre_pool,
                tc.tile_pool(name="h1", bufs=4) as h1_pool,
                tc.tile_pool(name="h2", bufs=6) as h2_pool,
                tc.tile_pool(name="sigp", bufs=3) as sig_pool,
                tc.tile_pool(name="l2ps", bufs=3, space="PSUM") as l2_pool,
                tc.tile_pool(name="l3ps", bufs=2, space="PSUM") as l3_pool,
            ):
                h2_tiles = {}     # sb -> h2 AP
                psum3_tiles = {}  # pair -> psum3 AP
                h1_bufs = {}      # quad -> h1 AP
                h1pre_bufs = {}

                def emit_s1_tt(sb2, nsb=2, eng=None):
                    # S1 chunks cover nsb sbs each: small chunks early (so the
                    # pipeline fills fast), 4-sb chunks mid-stream (amortizes
                    # the ~140ns/op DVE startup cost).
                    quad, h4 = divmod(sb2, 2)
                    if h4 == 0:
                        h1pre_bufs[quad] = h1pre_pool.tile(
                            [128, 4 * COLS], BF16, tag="h1pre", name="h1pre_t"
                        )[:]
                        h1_bufs[quad] = h1_pool.tile(
                            [128, 4 * COLS], BF16, tag="h1", name="h1_t"
                        )[:]
                    h1pre = h1pre_bufs[quad]
                    a_in = (
                        A2dup[:, 128 * sb2 : 128 * sb2 + 64 * nsb]
                        .rearrange("p (s i two) -> p s i two", s=nsb, two=2)
                        .unsqueeze(3)
                        .broadcast_to([128, nsb, 32, 16, 2])
                    )
                    b_in = (
                        B2s[:, 64 * sb2 : 64 * sb2 + 32 * nsb]
                        .rearrange("p (s jh jl) -> p s jh jl", s=nsb, jl=2)
                        .unsqueeze(2)
                        .broadcast_to([128, nsb, 32, 16, 2])
                    )
                    h1pre_v = h1pre[
                        :, 2 * COLS * h4 : 2 * COLS * h4 + nsb * COLS
                    ].rearrange("p (s i jh jl) -> p s i jh jl", s=nsb, i=32, jl=2)
                    (eng or nc.vector).tensor_add(h1pre_v, a_in, b_in)

                def emit_s1_relu(sb2, nsb=2):
                    quad, h4 = divmod(sb2, 2)
                    cs = slice(2 * COLS * h4, 2 * COLS * h4 + nsb * COLS)
                    nc.vector.tensor_scalar_max(
                        h1_bufs[quad][:, cs], h1pre_bufs[quad][:, cs], 0.0
                    )

                def emit_s1_single(sb):
                    # 1-sb S1 (used for the last two sbs): halves the latency
                    # from "h1 ready" to the tail L2->evict->L3->sigmoid chain.
                    quad, k4 = divmod(sb, 4)
                    h1pre = h1pre_bufs[quad]
                    a_in = (
                        A2dup[:, 64 * sb : 64 * sb + 64]
                        .rearrange("p (i two) -> p i two", two=2)
                        .unsqueeze(2)
                        .broadcast_to([128, 32, 16, 2])
                    )
                    b_in = (
                        B2s[:, 32 * sb : 32 * sb + 32]
                        .rearrange("p (jh jl) -> p jh jl", jl=2)
                        .unsqueeze(1)
                        .broadcast_to([128, 32, 16, 2])
                    )
                    cs = slice(COLS * k4, COLS * k4 + COLS)
                    h1pre_v = h1pre[:, cs].rearrange(
                        "p (i jh jl) -> p i jh jl", i=32, jl=2
                    )
                    nc.vector.tensor_add(h1pre_v, a_in, b_in)
                    nc.vector.tensor_scalar_max(h1_bufs[quad][:, cs], h1pre[:, cs], 0.0)

                def emit_l2_s2(sb, hk):
                    # K=128 M=128 block-diagonal W2: one matmul per 512 cols.
                    psum2 = l2_pool.tile([128, COLS], FP32, tag="l2", name="psum2_t")[:]
                    nc.tensor.matmul(psum2[:, 0:512], W2blk, hk[:, 0:512])
                    nc.tensor.matmul(psum2[:, 512:1024], W2blk, hk[:, 512:1024])
                    h2 = h2_pool.tile([128, COLS], BF16, tag="h2", name="h2_t")[:]
                    if sb in H2EV_DVE:
                        nc.vector.tensor_scalar(h2, psum2, bias2, 0.0, add, max_)
                    else:
                        nc.scalar.activation(h2, psum2, AF.Relu, bias=bias2)
                    h2_tiles[sb] = h2

                def emit_l3(sb):
                    # K=128 M=32: W3v packs W3 for both l-block parities into
                    # rows 0:8 / 16:24 of a 32-wide M block; tile_position col
                    # offsets place the pair's four (sb,chunk) units in one
                    # [128, 512] psum bank.
                    pair, k = divmod(sb, 2)
                    if k == 0:
                        psum3_tiles[pair] = l3_pool.tile(
                            [128, 512], FP32, tag="l3", name="psum3_t"
                        )[:]
                    psum3 = psum3_tiles[pair]
                    h2 = h2_tiles.pop(sb)
                    for ch in (0, 1):
                        u = 2 * k + ch
                        nc.tensor.matmul(
                            psum3[32 * u : 32 * u + 32, :],
                            W3v,
                            h2[:, 512 * ch : 512 * ch + 512],
                            tile_position=(0, 32 * u),
                        )

                def emit_sigmoid_dma(pair):
                    sig = sig_pool.tile([128, 512], BF16, tag="sig", name="sig_t")[:]
                    psum3 = psum3_tiles.pop(pair)
                    nc.scalar.activation(sig, psum3, AF.Sigmoid, bias=bias3)
                    if pair == NPAIR - 1:
                        # Split the last transfer so the drain tail halves.
                        nc.sync.dma_start(out=d_out[pair, 0:64], in_=sig[0:64])
                        nc.gpsimd.dma_start(out=d_out[pair, 64:128], in_=sig[64:128])
                    else:
                        eng = nc.sync if pair % 2 == 0 else nc.gpsimd
                        eng.dma_start(out=d_out[pair], in_=sig)

                # Software pipeline, deliberately deep so no strict-FIFO queue
                # ever stalls at its head: S1 runs two quads (8 sbs) ahead of
                # L2, in per-2sb chunks; L3 lags L2 by two sbs (its h2 is long
                # since evicted); the sigmoid lags its pair's last L3 too.
                # S1 emission plan: 2-sb chunks up front (fast pipeline fill),
                # 4-sb (full-quad) chunks mid-stream, per-sb ops at the tail.
                s1_plan = {2: 4, 6: 6, 10: 8, 14: 10, 18: 12}
                for sb2 in range(4):
                    emit_s1_tt(sb2)
                    emit_s1_relu(sb2)
                for sb in range(NSB):
                    quad, k4 = divmod(sb, 4)
                    if sb in s1_plan:
                        emit_s1_tt(s1_plan[sb], nsb=4)
                        emit_s1_relu(s1_plan[sb], nsb=4)
                    elif sb == 22:
                        emit_s1_tt(14)
                        emit_s1_relu(14)
                    elif sb == 24:
                        emit_s1_single(30)
                        emit_s1_single(31)
                    emit_l2_s2(sb, h1_bufs[quad][:, COLS * k4 : COLS * k4 + COLS])
                    if sb >= 2:
                        emit_l3(sb - 2)
                    if sb % 2 == 1 and sb >= 3:  # pair (sb-3)//2 L3'd at sb-1
                        emit_sigmoid_dma((sb - 3) // 2)
                emit_l3(NSB - 2)
                emit_l3(NSB - 1)
                emit_sigmoid_dma(NPAIR - 1)

    nc.compile()
    input_names = ["statesQ", "Wl1", "W2blk", "W3v", "biases"]
    return nc, input_names


def get_program():
    global _PROGRAM
    if _PROGRAM is None:
        _PROGRAM = _build_program()
    return _PROGRAM


def make_inputs(states, W1, b1, W2, b2, W3, b3):
    """Host-side prep: per-core statesQ + shared packed weights/biases."""
    states = np.asarray(states, np.float32)
    W1 = np.asarray(W1, np.float32)
    W2 = np.asarray(W2, np.float32)
    W3 = np.asarray(W3, np.float32)
    b1 = np.asarray(b1, np.float32)
    b2 = np.asarray(b2, np.float32)
    b3 = np.asarray(b3, np.float32)

    # Wl1 [32, 256]: cols 0:128 = K=32 parity-blockdiag lhsT for A2
    # (rows 0:16 -> outs 0:64 even, rows 16:32 -> outs 64:128 odd),
    # cols 128:256 = same structure for B2 (second half of W1).
    Wl1 = np.zeros((32, 256), NP_BF16)
    Wl1[0:16, 0:64] = W1[:D].astype(NP_BF16)
    Wl1[16:32, 64:128] = W1[:D].astype(NP_BF16)
    Wl1[0:16, 128:192] = W1[D:].astype(NP_BF16)
    Wl1[16:32, 192:256] = W1[D:].astype(NP_BF16)

    W2blk = np.zeros((128, 128), NP_BF16)
    W2blk[0:64, 0:64] = W2.astype(NP_BF16)
    W2blk[64:128, 64:128] = W2.astype(NP_BF16)

    # W3v: K=128 M=32 weight for L3 — even-parity h2 (parts 0:64) feeds out
    # rows 0:8, odd-parity (64:128) feeds rows 16:24.
    W3v = np.zeros((128, 32), NP_BF16)
    W3v[0:64, 0:8] = W3.astype(NP_BF16)
    W3v[64:128, 16:24] = W3.astype(NP_BF16)

    biases = np.zeros((128, 3), np.float32)
    biases[:, 0] = np.tile(b1, 2)
    biases[:, 1] = np.tile(b2, 2)
    b3pat = np.zeros(32, np.float32)
    b3pat[0:8] = b3
    b3pat[16:24] = b3
    biases[:, 2] = np.tile(b3pat, 4)

    shared = {"Wl1": Wl1, "W2blk": W2blk, "W3v": W3v, "biases": biases}

    in_maps = []
    for c in range(NCORES):
        # statesT[d, 32*l + i] = states[c, l, i, d]
        statesT = states[c].reshape(L * N, D).T.astype(NP_BF16)
        sQ = np.zeros((32, 2048), NP_BF16)
        sQ[0:16] = statesT
        sQ[16:32, : 2048 - 32] = statesT[:, 32:]
        in_maps.append({"statesQ": sQ, **shared})
    return in_maps


def decode_output(raw):
    """Invert the device output layout -> [L, N, N, F] for one core.

    raw: [NPAIR, 128, 512] bf16. Partitions: u = 2*s + ch in blocks of 32
    (s = sb within pair, ch = column chunk), rows 0:8 = even l-block
    features, 16:24 = odd. col c: pair-col = 512*ch + c = 32*i + j.
    l = 4*pair + 2*s + parity.
    """
    raw = raw.astype(np.float32)
    ov = raw.reshape(NPAIR, 2, 2, 32, 512)       # [pair, s, ch, row, c]
    ev = ov[:, :, :, 0:8, :]                     # even parity
    od = ov[:, :, :, 16:24, :]                   # odd parity
    st = np.stack([ev, od], axis=3)              # [pair, s, ch, par, f, c]
    st = st.transpose(0, 1, 3, 2, 5, 4)          # [pair, s, par, ch, c, f]
    return np.ascontiguousarray(st.reshape(L, N * N, F)).reshape(L, N, N, F)


def _ensure_ntff_hook():
    """Best-effort shim for the missing antenv.axon_hooks module so
    run_bass_kernel_spmd(trace=True) can capture NTFF profiles under axon."""
    import types

    try:
        from antenv.axon_hooks import get_axon_ntff_profile_hook  # noqa: F401
        return
    except ImportError:
        pass
    try:
        if "/root/.axon_site" not in sys.path:
            sys.path.insert(0, "/root/.axon_site")
        from trn_agent_boot.trn_boot import _ntff_profile_via_ctypes

        hook = _ntff_profile_via_ctypes("/opt/axon/libaxon_pjrt.so")
        import antenv

        mod = types.ModuleType("antenv.axon_hooks")
        mod._hook = hook
        mod.set_axon_ntff_profile_hook = lambda h: setattr(mod, "_hook", h)
        mod.get_axon_ntff_profile_hook = lambda: mod._hook
        sys.modules["antenv.axon_hooks"] = mod
        antenv.axon_hooks = mod
    except Exception as e:  # tracing is optional; never break the run
        print(f"ntff hook shim failed: {e}", file=sys.stderr)


def kernel(states, W1, b1, W2, b2, W3, b3):
    global LAST_RESULT
    nc, _ = get_program()
    if os.environ.get("KERNEL_TRACE"):
        _ensure_ntff_hook()
    in_maps = make_inputs(states, W1, b1, W2, b2, W3, b3)
    res = run_bass_kernel_spmd(
        nc,
        in_maps,
        core_ids=list(range(NCORES)),
        trace=bool(os.environ.get("KERNEL_TRACE")),
    )
    LAST_RESULT = res
    out = np.empty((B, L, N, N, F), np.float32)
    for c in range(NCORES):
        out[c] = decode_output(res.results[c]["out"])
    return out


# revision 45
# speedup vs baseline: 1.0291x; 1.0284x over previous
"""Trainium2 Bass kernel for pairwise-MLP GNN message passing.

Computation (per batch b, position l):
    x[i,j] = concat(states[l,i], states[l,j])           # [N,N,2D]
    out    = sigmoid(MLP(x))                            # [N,N,8], MLP: 32->64->64->8

Factorization used on device: the first linear layer splits into
A = states @ W1[:D] + b1 and B = states @ W1[D:], so
h1[i,j] = relu(A[i] + B[j]) — the N^2 expansion happens as a cheap
broadcast add on the vector engine instead of an N^2-row matmul.

Sharding: data-parallel over batch, core c <- batch c (8 cores, B=8).

Device design (per core, L=64 l-blocks; 2 l-blocks = 1 "sb"):
  - features on partitions: partitions 0:64 = even l-block of the sb,
    64:128 = odd l-block (host-side shifted copy of states^T feeding
    block-diagonal-packed matmuls); pair columns col = 32*i + j.
  - S1 (broadcast add + relu) runs on VectorE (tensor_tensor at 2x mode
    via the duplicated-A operand + tensor_scalar relu at 4x) in 2-sb
    chunks at the head (fast pipeline fill), 4-sb chunks mid-stream
    (amortize op startup), and per-sb ops at the tail (short final
    L2->evict->L3->sigmoid chain). A2dup evictions run on ScalarE and
    B2 on the (otherwise idle) DVE during L1 so S1 starts early.
  - L2: ONE K=128/M=128 block-diagonal matmul per 512 cols (measured:
    matmul cost is ~216ns per 512-col stream regardless of K/M in
    {64,128}) — 2 matmuls/sb, half the PE time of quadrant packing.
  - h2 eviction [128, 1024] per sb: ScalarE activation(Relu, bias) for
    most sbs, DVE tensor_scalar for the sbs in H2EV_DVE (engine load
    balance; DVE's S1 stream finishes ~2/3 through the kernel).
  - L3: K=128/M=32 matmuls with W3 packed at partition offsets 0:8
    (even l-block) / 16:24 (odd); tile_position col offsets 0/32/64/96
    place the four (sb-in-pair, col-chunk) units of a pair in one
    [128, 512] psum bank -> ONE [128, 512] sigmoid per pair (4x fewer
    ScalarE sigmoid columns than a [128,1024]-per-pair layout).
  - Output: one [128, 512] bf16 DMA per pair (dispatch alternates
    sync/gpsimd queues); host decodes the 32 valid rows.
"""

import os
import sys

import numpy as np

for _p in ("/opt/trn_rl_repo", "/root/.axon_site/_ro/trn_rl_repo"):
    if os.path.isdir(_p) and _p not in sys.path:
        sys.path.insert(0, _p)

from concourse import bacc, mybir, tile
from concourse.bass_utils import run_bass_kernel_spmd

B, L, N, D = 8, 64, 32, 16
H = 64            # hidden width (h1 and h2)
F = 8             # out_dim
NCORES = 8
NSB = L // 2      # 32 superblocks per core
NPAIR = NSB // 2  # 16 pairs per core
COLS = N * N      # 1024 pair columns per l-block

# sbs whose h2 eviction runs on VectorE (rest on ScalarE); spread through
# the mid/late phase so the DVE's S1 stream stays ahead of L2.
H2EV_DVE = frozenset({13, 16, 19, 22, 25, 28, 31})

FP32 = mybir.dt.float32
BF16 = mybir.dt.bfloat16
NP_BF16 = mybir.dt.np(BF16)

_PROGRAM = None  # (nc, input_names)
LAST_RESULT = None  # BassKernelResults of the most recent kernel() call


def _build_program():
    nc = bacc.Bacc("TRN2", target_bir_lowering=False, debug=False)

    d_statesQ = nc.dram_tensor("statesQ", [32, 2048], BF16, kind="ExternalInput").ap()
    d_Wl1 = nc.dram_tensor("Wl1", [32, 256], BF16, kind="ExternalInput").ap()
    d_W2blk = nc.dram_tensor("W2blk", [128, 128], BF16, kind="ExternalInput").ap()
    d_W3v = nc.dram_tensor("W3v", [128, 32], BF16, kind="ExternalInput").ap()
    d_biases = nc.dram_tensor("biases", [128, 3], FP32, kind="ExternalInput").ap()
    d_out = nc.dram_tensor("out", [NPAIR, 128, 512], BF16, kind="ExternalOutput").ap()

    add = mybir.AluOpType.add
    max_ = mybir.AluOpType.max
    AF = mybir.ActivationFunctionType

    with tile.TileContext(nc) as tc:
        with tc.tile_pool(name="const", bufs=1) as const_pool:
            statesQ = const_pool.tile([32, 2048], BF16, name="statesQ_t")[:]
            Wl1 = const_pool.tile([32, 256], BF16, name="Wl1_t")[:]
            W2blk = const_pool.tile([128, 128], BF16, name="W2blk_t")[:]
            W3v = const_pool.tile([128, 32], BF16, name="W3v_t")[:]
            biases = const_pool.tile([128, 3], FP32, name="biases_t")[:]
            bias1 = biases[:, 0:1]
            bias2 = biases[:, 1:2]
            bias3 = biases[:, 2:3]
            A2dup = const_pool.tile([128, 2 * COLS], BF16, name="A2dup_t")[:]
            B2s = const_pool.tile([128, COLS], BF16, name="B2s_t")[:]

            # Parallel dispatch across queues: serial dma_start dispatch costs
            # ~700ns each and would otherwise delay the L1 matmuls.
            nc.sync.dma_start(out=statesQ[0:16], in_=d_statesQ[0:16])
            nc.gpsimd.dma_start(out=statesQ[16:32], in_=d_statesQ[16:32])
            nc.scalar.dma_start(out=Wl1, in_=d_Wl1)
            nc.scalar.dma_start(out=biases, in_=d_biases)
            nc.sync.dma_start(out=W2blk, in_=d_W2blk)
            nc.gpsimd.dma_start(out=W3v, in_=d_W3v)


            # Warm the ACT table with the sigmoid set up front: Identity/Relu
            # are filler functions present in every set, so this is the only
            # ACT_TABLE_LOAD the kernel pays. Reads the framework's memset
            # const tile, NOT an input, so the load doesn't wait on any DMA.
            sigwarm = const_pool.tile([128, 1], FP32, name="sigwarm_t")[:]
            nc.scalar.activation(sigwarm, nc.const_aps.aps[(FP32, 0.0)], AF.Sigmoid)

            # ---- Layer 1: A2/B2 = per-agent halves of the first linear layer.
            # A2[p, 32*sb + i]: p<64 -> even l-block (2sb), p>=64 -> odd (2sb+1)
            # — K=32 block-diagonal over the parities (statesQ rows 0:16 even,
            # 16:32 odd-shifted), so L1 is 4 matmuls. Column-chunk (sbh) outer
            # so the first 512 A2/B2 cols finish first and their evictions
            # (split below) unblock the DVE's S1 stream earlier.
            with tc.tile_pool(name="abps", bufs=1, space="PSUM") as ab_pool:
                A2ps = ab_pool.tile([128, COLS], FP32, tag="a2", name="A2ps_t")[:]
                B2ps = ab_pool.tile([128, COLS], FP32, tag="b2", name="B2ps_t")[:]
                rhs = statesQ.rearrange("p (s c) -> p s c", s=32)
                for sbh in (0, 1):
                    for w_lo, ps in ((0, A2ps), (128, B2ps)):
                        nc.tensor.matmul(
                            ps[:, 512 * sbh : 512 * sbh + 512],
                            Wl1[:, w_lo : w_lo + 128],
                            rhs[:, 16 * sbh : 16 * sbh + 16, 0:32],
                        )
                # Evict A2 twice (duplicated pairs so the later broadcast add
                # keeps an innermost unit stride), folding in b1; B2 plain.
                # All on ScalarE (the DVE starts S1 with no preliminaries),
                # small head chunks first: S1 chunk 0 only needs cols 0:256.
                dupview = A2dup.rearrange("p (c two) -> p two c", two=2)
                for c0, c1 in ((0, 256), (256, 512), (512, 1024)):
                    nc.scalar.activation(
                        dupview[:, 0, c0:c1], A2ps[:, c0:c1], AF.Identity, bias=bias1
                    )
                    nc.scalar.activation(
                        dupview[:, 1, c0:c1], A2ps[:, c0:c1], AF.Identity, bias=bias1
                    )
                    # B2 on the DVE: it idles here anyway until A2dup lands.
                    nc.vector.tensor_scalar_add(B2s[:, c0:c1], B2ps[:, c0:c1], 0.0)

            with (
                tc.tile_pool(name="h1pre", bufs=2) as h1pre_pool,
                tc.tile_pool(name="h1", bufs=4) as h1_pool,
                tc.tile_pool(name="h2", bufs=6) as h2_pool,
                tc.tile_pool(name="sigp", bufs=3) as sig_pool,
                tc.tile_pool(name="l2ps", bufs=3, space="PSUM") as l2_pool,
                tc.tile_pool(name="l3ps", bufs=2, space="PSUM") as l3_pool,
            ):
                h2_tiles = {}     # sb -> h2 AP
                psum3_tiles = {}  # pair -> psum3 AP
                h1_bufs = {}      # quad -> h1 AP
                h1pre_bufs = {}

                def emit_s1_tt(sb2, nsb=2, eng=None):
                    # S1 chunks cover nsb sbs each: small chunks early (so the
                    # pipeline fills fast), 4-sb chunks mid-stream (amortizes
                    # the ~140ns/op DVE startup cost).
                    quad, h4 = divmod(sb2, 2)
                    if h4 == 0:
                        h1pre_bufs[quad] = h1pre_pool.tile(
                            [128, 4 * COLS], BF16, tag="h1pre", name="h1pre_t"
                        )[:]
                        h1_bufs[quad] = h1_pool.tile(
                            [128, 4 * COLS], BF16, tag="h1", name="h1_t"
                        )[:]
                    h1pre = h1pre_bufs[quad]
                    a_in = (
                        A2dup[:, 128 * sb2 : 128 * sb2 + 64 * nsb]
                        .rearrange("p (s i two) -> p s i two", s=nsb, two=2)
                        .unsqueeze(3)
                        .broadcast_to([128, nsb, 32, 16, 2])
                    )
                    b_in = (
                        B2s[:, 64 * sb2 : 64 * sb2 + 32 * nsb]
                        .rearrange("p (s jh jl) -> p s jh jl", s=nsb, jl=2)
                        .unsqueeze(2)
                        .broadcast_to([128, nsb, 32, 16, 2])
                    )
                    h1pre_v = h1pre[
                        :, 2 * COLS * h4 : 2 * COLS * h4 + nsb * COLS
                    ].rearrange("p (s i jh jl) -> p s i jh jl", s=nsb, i=32, jl=2)
                    (eng or nc.vector).tensor_add(h1pre_v, a_in, b_in)

                def emit_s1_relu(sb2, nsb=2):
                    quad, h4 = divmod(sb2, 2)
                    cs = slice(2 * COLS * h4, 2 * COLS * h4 + nsb * COLS)
                    nc.vector.tensor_scalar_max(
                        h1_bufs[quad][:, cs], h1pre_bufs[quad][:, cs], 0.0
                    )

                def emit_s1_single(sb):
                    # 1-sb S1 (used for the last two sbs): halves the latency
                    # from "h1 ready" to the tail L2->evict->L3->sigmoid chain.
                    quad, k4 = divmod(sb, 4)
                    h1pre = h1pre_bufs[quad]
                    a_in = (
                        A2dup[:, 64 * sb : 64 * sb + 64]
                        .rearrange("p (i two) -> p i two", two=2)
                        .unsqueeze(2)
                        .broadcast_to([128, 32, 16, 2])
                    )
                    b_in = (
                        B2s[:, 32 * sb : 32 * sb + 32]
                        .rearrange("p (jh jl) -> p jh jl", jl=2)
                        .unsqueeze(1)
                        .broadcast_to([128, 32, 16, 2])
                    )
                    cs = slice(COLS * k4, COLS * k4 + COLS)
                    h1pre_v = h1pre[:, cs].rearrange(
                        "p (i jh jl) -> p i jh jl", i=32, jl=2
                    )
                    nc.vector.tensor_add(h1pre_v, a_in, b_in)
                    nc.vector.tensor_scalar_max(h1_bufs[quad][:, cs], h1pre[:, cs], 0.0)

                def emit_l2_s2(sb, hk):
                    # K=128 M=128 block-diagonal W2: one matmul per 512 cols.
                    psum2 = l2_pool.tile([128, COLS], FP32, tag="l2", name="psum2_t")[:]
                    nc.tensor.matmul(psum2[:, 0:512], W2blk, hk[:, 0:512])
                    nc.tensor.matmul(psum2[:, 512:1024], W2blk, hk[:, 512:1024])
                    h2 = h2_pool.tile([128, COLS], BF16, tag="h2", name="h2_t")[:]
                    if sb in H2EV_DVE:
                        nc.vector.tensor_scalar(h2, psum2, bias2, 0.0, add, max_)
                    else:
                        nc.scalar.activation(h2, psum2, AF.Relu, bias=bias2)
                    h2_tiles[sb] = h2

                def emit_l3(sb):
                    # K=128 M=32: W3v packs W3 for both l-block parities into
                    # rows 0:8 / 16:24 of a 32-wide M block; tile_position col
                    # offsets place the pair's four (sb,chunk) units in one
                    # [128, 512] psum bank.
                    pair, k = divmod(sb, 2)
                    if k == 0:
                        psum3_tiles[pair] = l3_pool.tile(
                            [128, 512], FP32, tag="l3", name="psum3_t"
                        )[:]
                    psum3 = psum3_tiles[pair]
                    h2 = h2_tiles.pop(sb)
                    for ch in (0, 1):
                        u = 2 * k + ch
                        nc.tensor.matmul(
                            psum3[32 * u : 32 * u + 32, :],
                            W3v,
                            h2[:, 512 * ch : 512 * ch + 512],
                            tile_position=(0, 32 * u),
                        )

                def emit_sigmoid_dma(pair):
                    sig = sig_pool.tile([128, 512], BF16, tag="sig", name="sig_t")[:]
                    psum3 = psum3_tiles.pop(pair)
                    nc.scalar.activation(sig, psum3, AF.Sigmoid, bias=bias3)
                    if pair == NPAIR - 1:
                        # Split the last transfer so the drain tail halves.
                        nc.sync.dma_start(out=d_out[pair, 0:64], in_=sig[0:64])
                        nc.gpsimd.dma_start(out=d_out[pair, 64:128], in_=sig[64:128])
                    else:
                        eng = nc.sync if pair % 2 == 0 else nc.gpsimd
                        eng.dma_start(out=d_out[pair], in_=sig)

                # Software pipeline, deliberately deep so no strict-FIFO queue
                # ever stalls at its head: S1 runs two quads (8 sbs) ahead of
                # L2, in per-2sb chunks; L3 lags L2 by two sbs (its h2 is long
                # since evicted); the sigmoid lags its pair's last L3 too.
                # S1 emission plan: 2-sb chunks up front (fast pipeline fill),
                # 4-sb (full-quad) chunks mid-stream, per-sb ops at the tail.
                s1_plan = {2: 4, 6: 6, 10: 8, 14: 10, 18: 12}
                for sb2 in range(4):
                    emit_s1_tt(sb2)
                    emit_s1_relu(sb2)
                for sb in range(NSB):
                    quad, k4 = divmod(sb, 4)
                    if sb in s1_plan:
                        emit_s1_tt(s1_plan[sb], nsb=4)
                        emit_s1_relu(s1_plan[sb], nsb=4)
                    elif sb == 22:
                        emit_s1_tt(14)
                        emit_s1_relu(14)
                    elif sb == 24:
                        emit_s1_single(30)
                        emit_s1_single(31)
                    emit_l2_s2(sb, h1_bufs[quad][:, COLS * k4 : COLS * k4 + COLS])
                    if sb >= 2:
                        emit_l3(sb - 2)
                    if sb % 2 == 1 and sb >= 3:  # pair (sb-3)//2 L3'd at sb-1
                        emit_sigmoid_dma((sb - 3) // 2)
                emit_l3(NSB - 2)
                emit_l3(NSB - 1)
                emit_sigmoid_dma(NPAIR - 1)

    nc.compile()
    input_names = ["statesQ", "Wl1", "W2blk", "W3v", "biases"]
    return nc, input_names


def get_program():
    global _PROGRAM
    if _PROGRAM is None:
        _PROGRAM = _build_program()
    return _PROGRAM


def make_inputs(states, W1, b1, W2, b2, W3, b3):
    """Host-side prep: per-core statesQ + shared packed weights/biases."""
    states = np.asarray(states, np.float32)
    W1 = np.asarray(W1, np.float32)
    W2 = np.asarray(W2, np.float32)
    W3 = np.asarray(W3, np.float32)
    b1 = np.asarray(b1, np.float32)
    b2 = np.asarray(b2, np.float32)
    b3 = np.asarray(b3, np.float32)

    # Wl1 [32, 256]: cols 0:128 = K=32 parity-blockdiag lhsT for A2
    # (rows 0:16 -> outs 0:64 even, rows 16:32 -> outs 64:128 odd),
    # cols 128:256 = same structure for B2 (second half of W1).
    Wl1 = np.zeros((32, 256), NP_BF16)
    Wl1[0:16, 0:64] = W1[:D].astype(NP_BF16)
    Wl1[16:32, 64:128] = W1[:D].astype(NP_BF16)
    Wl1[0:16, 128:192] = W1[D:].astype(NP_BF16)
    Wl1[16:32, 192:256] = W1[D:].astype(NP_BF16)

    W2blk = np.zeros((128, 128), NP_BF16)
    W2blk[0:64, 0:64] = W2.astype(NP_BF16)
    W2blk[64:128, 64:128] = W2.astype(NP_BF16)

    # W3v: K=128 M=32 weight for L3 — even-parity h2 (parts 0:64) feeds out
    # rows 0:8, odd-parity (64:128) feeds rows 16:24.
    W3v = np.zeros((128, 32), NP_BF16)
    W3v[0:64, 0:8] = W3.astype(NP_BF16)
    W3v[64:128, 16:24] = W3.astype(NP_BF16)

    biases = np.zeros((128, 3), np.float32)
    biases[:, 0] = np.tile(b1, 2)
    biases[:, 1] = np.tile(b2, 2)
    b3pat = np.zeros(32, np.float32)
    b3pat[0:8] = b3
    b3pat[16:24] = b3
    biases[:, 2] = np.tile(b3pat, 4)

    shared = {"Wl1": Wl1, "W2blk": W2blk, "W3v": W3v, "biases": biases}

    in_maps = []
    for c in range(NCORES):
        # statesT[d, 32*l + i] = states[c, l, i, d]
        statesT = states[c].reshape(L * N, D).T.astype(NP_BF16)
        sQ = np.zeros((32, 2048), NP_BF16)
        sQ[0:16] = statesT
        sQ[16:32, : 2048 - 32] = statesT[:, 32:]
        in_maps.append({"statesQ": sQ, **shared})
    return in_maps


def decode_output(raw):
    """Invert the device output layout -> [L, N, N, F] for one core.

    raw: [NPAIR, 128, 512] bf16. Partitions: u = 2*s + ch in blocks of 32
    (s = sb within pair, ch = column chunk), rows 0:8 = even l-block
    features, 16:24 = odd. col c: pair-col = 512*ch + c = 32*i + j.
    l = 4*pair + 2*s + parity.
    """
    raw = raw.astype(np.float32)
    ov = raw.reshape(NPAIR, 2, 2, 32, 512)       # [pair, s, ch, row, c]
    ev = ov[:, :, :, 0:8, :]                     # even parity
    od = ov[:, :, :, 16:24, :]                   # odd parity
    st = np.stack([ev, od], axis=3)              # [pair, s, ch, par, f, c]
    st = st.transpose(0, 1, 3, 2, 5, 4)          # [pair, s, par, ch, c, f]
    return np.ascontiguousarray(st.reshape(L, N * N, F)).reshape(L, N, N, F)


def _ensure_ntff_hook():
    """Best-effort shim for the missing antenv.axon_hooks module so
    run_bass_kernel_spmd(trace=True) can capture NTFF profiles under axon."""
    import types

    try:
        from antenv.axon_hooks import get_axon_ntff_profile_hook  # noqa: F401
        return
    except ImportError:
        pass
    try:
        if "/root/.axon_site" not in sys.path:
            sys.path.insert(0, "/root/.axon_site")
        from trn_agent_boot.trn_boot import _ntff_profile_via_ctypes

        hook = _ntff_profile_via_ctypes("/opt/axon/libaxon_pjrt.so")
        import antenv

        mod = types.ModuleType("antenv.axon_hooks")
        mod._hook = hook
        mod.set_axon_ntff_profile_hook = lambda h: setattr(mod, "_hook", h)
        mod.get_axon_ntff_profile_hook = lambda: mod._hook
        sys.modules["antenv.axon_hooks"] = mod
        antenv.axon_hooks = mod
    except Exception as e:  # tracing is optional; never break the run
        print(f"ntff hook shim failed: {e}", file=sys.stderr)


def kernel(states, W1, b1, W2, b2, W3, b3):
    global LAST_RESULT
    nc, _ = get_program()
    if os.environ.get("KERNEL_TRACE"):
        _ensure_ntff_hook()
    in_maps = make_inputs(states, W1, b1, W2, b2, W3, b3)
    res = run_bass_kernel_spmd(
        nc,
        in_maps,
        core_ids=list(range(NCORES)),
        trace=bool(os.environ.get("KERNEL_TRACE")),
    )
    LAST_RESULT = res
    out = np.empty((B, L, N, N, F), np.float32)
    for c in range(NCORES):
        out[c] = decode_output(res.results[c]["out"])
    return out


# revision 47
# speedup vs baseline: 1.0336x; 1.0044x over previous
"""Trainium2 Bass kernel for pairwise-MLP GNN message passing.

Computation (per batch b, position l):
    x[i,j] = concat(states[l,i], states[l,j])           # [N,N,2D]
    out    = sigmoid(MLP(x))                            # [N,N,8], MLP: 32->64->64->8

Factorization used on device: the first linear layer splits into
A = states @ W1[:D] + b1 and B = states @ W1[D:], so
h1[i,j] = relu(A[i] + B[j]) — the N^2 expansion happens as a cheap
broadcast add on the vector engine instead of an N^2-row matmul.

Sharding: data-parallel over batch, core c <- batch c (8 cores, B=8).

Device design (per core, L=64 l-blocks; 2 l-blocks = 1 "sb"):
  - features on partitions: partitions 0:64 = even l-block of the sb,
    64:128 = odd l-block (host-side shifted copy of states^T feeding
    block-diagonal-packed matmuls); pair columns col = 32*i + j.
  - S1 (broadcast add + relu) runs on VectorE (tensor_tensor at 2x mode
    via the duplicated-A operand + tensor_scalar relu at 4x) in 2-sb
    chunks at the head (fast pipeline fill), 4-sb chunks mid-stream
    (amortize op startup), and per-sb ops at the tail (short final
    L2->evict->L3->sigmoid chain). A2dup evictions run on ScalarE and
    B2 on the (otherwise idle) DVE during L1 so S1 starts early.
  - L2: ONE K=128/M=128 block-diagonal matmul per 512 cols (measured:
    matmul cost is ~216ns per 512-col stream regardless of K/M in
    {64,128}) — 2 matmuls/sb, half the PE time of quadrant packing.
  - h2 eviction [128, 1024] per sb: ScalarE activation(Relu, bias) for
    most sbs, DVE tensor_scalar for the sbs in H2EV_DVE (engine load
    balance; DVE's S1 stream finishes ~2/3 through the kernel).
  - L3: K=128/M=32 matmuls with W3 packed at partition offsets 0:8
    (even l-block) / 16:24 (odd); tile_position col offsets 0/32/64/96
    place the four (sb-in-pair, col-chunk) units of a pair in one
    [128, 512] psum bank -> ONE [128, 512] sigmoid per pair (4x fewer
    ScalarE sigmoid columns than a [128,1024]-per-pair layout).
  - Output: one [128, 512] bf16 DMA per pair (dispatch alternates
    sync/gpsimd queues); host decodes the 32 valid rows.
"""

import os
import sys

import numpy as np

for _p in ("/opt/trn_rl_repo", "/root/.axon_site/_ro/trn_rl_repo"):
    if os.path.isdir(_p) and _p not in sys.path:
        sys.path.insert(0, _p)

from concourse import bacc, mybir, tile
from concourse.bass_utils import run_bass_kernel_spmd

B, L, N, D = 8, 64, 32, 16
H = 64            # hidden width (h1 and h2)
F = 8             # out_dim
NCORES = 8
NSB = L // 2      # 32 superblocks per core
NPAIR = NSB // 2  # 16 pairs per core
COLS = N * N      # 1024 pair columns per l-block

# sbs whose h2 eviction runs on VectorE (rest on ScalarE); spread through
# the mid/late phase so the DVE's S1 stream stays ahead of L2.
H2EV_DVE = frozenset({13, 16, 19, 22, 25, 28, 31})

FP32 = mybir.dt.float32
BF16 = mybir.dt.bfloat16
NP_BF16 = mybir.dt.np(BF16)

_PROGRAM = None  # (nc, input_names)
LAST_RESULT = None  # BassKernelResults of the most recent kernel() call


def _build_program():
    nc = bacc.Bacc("TRN2", target_bir_lowering=False, debug=False)

    d_statesQ = nc.dram_tensor("statesQ", [32, 2048], BF16, kind="ExternalInput").ap()
    d_Wl1 = nc.dram_tensor("Wl1", [32, 256], BF16, kind="ExternalInput").ap()
    d_W2blk = nc.dram_tensor("W2blk", [128, 128], BF16, kind="ExternalInput").ap()
    d_W3v = nc.dram_tensor("W3v", [128, 32], BF16, kind="ExternalInput").ap()
    d_biases = nc.dram_tensor("biases", [128, 3], FP32, kind="ExternalInput").ap()
    d_out = nc.dram_tensor("out", [NPAIR, 128, 512], BF16, kind="ExternalOutput").ap()

    add = mybir.AluOpType.add
    max_ = mybir.AluOpType.max
    AF = mybir.ActivationFunctionType

    with tile.TileContext(nc) as tc:
        with tc.tile_pool(name="const", bufs=1) as const_pool:
            statesQ = const_pool.tile([32, 2048], BF16, name="statesQ_t")[:]
            Wl1 = const_pool.tile([32, 256], BF16, name="Wl1_t")[:]
            W2blk = const_pool.tile([128, 128], BF16, name="W2blk_t")[:]
            W3v = const_pool.tile([128, 32], BF16, name="W3v_t")[:]
            biases = const_pool.tile([128, 3], FP32, name="biases_t")[:]
            bias1 = biases[:, 0:1]
            bias2 = biases[:, 1:2]
            bias3 = biases[:, 2:3]
            A2dup = const_pool.tile([128, 2 * COLS], BF16, name="A2dup_t")[:]
            B2s = const_pool.tile([128, COLS], BF16, name="B2s_t")[:]

            # Parallel dispatch across queues: serial dma_start dispatch costs
            # ~700ns each and would otherwise delay the L1 matmuls.
            nc.sync.dma_start(out=statesQ[0:16], in_=d_statesQ[0:16])
            nc.gpsimd.dma_start(out=statesQ[16:32], in_=d_statesQ[16:32])
            nc.scalar.dma_start(out=Wl1, in_=d_Wl1)
            nc.scalar.dma_start(out=biases, in_=d_biases)
            nc.sync.dma_start(out=W2blk, in_=d_W2blk)
            nc.gpsimd.dma_start(out=W3v, in_=d_W3v)


            # Warm the ACT table with the sigmoid set up front: Identity/Relu
            # are filler functions present in every set, so this is the only
            # ACT_TABLE_LOAD the kernel pays. Reads the framework's memset
            # const tile, NOT an input, so the load doesn't wait on any DMA.
            sigwarm = const_pool.tile([128, 1], FP32, name="sigwarm_t")[:]
            nc.scalar.activation(sigwarm, nc.const_aps.aps[(FP32, 0.0)], AF.Sigmoid)

            # ---- Layer 1: A2/B2 = per-agent halves of the first linear layer.
            # A2[p, 32*sb + i]: p<64 -> even l-block (2sb), p>=64 -> odd (2sb+1)
            # — K=32 block-diagonal over the parities (statesQ rows 0:16 even,
            # 16:32 odd-shifted), so L1 is 4 matmuls. Column-chunk (sbh) outer
            # so the first 512 A2/B2 cols finish first and their evictions
            # (split below) unblock the DVE's S1 stream earlier.
            with tc.tile_pool(name="abps", bufs=1, space="PSUM") as ab_pool:
                A2ps = ab_pool.tile([128, COLS], FP32, tag="a2", name="A2ps_t")[:]
                B2ps = ab_pool.tile([128, COLS], FP32, tag="b2", name="B2ps_t")[:]
                rhs = statesQ.rearrange("p (s c) -> p s c", s=32)
                for sbh in (0, 1):
                    for w_lo, ps in ((0, A2ps), (128, B2ps)):
                        nc.tensor.matmul(
                            ps[:, 512 * sbh : 512 * sbh + 512],
                            Wl1[:, w_lo : w_lo + 128],
                            rhs[:, 16 * sbh : 16 * sbh + 16, 0:32],
                        )
                # Evict A2 twice (duplicated pairs so the later broadcast add
                # keeps an innermost unit stride), folding in b1; B2 plain.
                # All on ScalarE (the DVE starts S1 with no preliminaries),
                # small head chunks first: S1 chunk 0 only needs cols 0:256.
                dupview = A2dup.rearrange("p (c two) -> p two c", two=2)
                for c0, c1 in ((0, 256), (256, 512), (512, 1024)):
                    nc.scalar.activation(
                        dupview[:, 0, c0:c1], A2ps[:, c0:c1], AF.Identity, bias=bias1
                    )
                    nc.scalar.activation(
                        dupview[:, 1, c0:c1], A2ps[:, c0:c1], AF.Identity, bias=bias1
                    )
                    # B2 on the DVE: it idles here anyway until A2dup lands.
                    nc.vector.tensor_scalar_add(B2s[:, c0:c1], B2ps[:, c0:c1], 0.0)

            with (
                tc.tile_pool(name="h1pre", bufs=2) as h1pre_pool,
                tc.tile_pool(name="h1", bufs=4) as h1_pool,
                tc.tile_pool(name="h2", bufs=6) as h2_pool,
                tc.tile_pool(name="sigp", bufs=3) as sig_pool,
                tc.tile_pool(name="l2ps", bufs=3, space="PSUM") as l2_pool,
                tc.tile_pool(name="l3ps", bufs=2, space="PSUM") as l3_pool,
            ):
                h2_tiles = {}     # sb -> h2 AP
                psum3_tiles = {}  # pair -> psum3 AP
                h1_bufs = {}      # quad -> h1 AP
                h1pre_bufs = {}

                def emit_s1_tt(sb2, nsb=2, eng=None):
                    # S1 chunks cover nsb sbs each: small chunks early (so the
                    # pipeline fills fast), 4-sb chunks mid-stream (amortizes
                    # the ~140ns/op DVE startup cost).
                    quad, h4 = divmod(sb2, 2)
                    if h4 == 0:
                        h1pre_bufs[quad] = h1pre_pool.tile(
                            [128, 4 * COLS], BF16, tag="h1pre", name="h1pre_t"
                        )[:]
                        h1_bufs[quad] = h1_pool.tile(
                            [128, 4 * COLS], BF16, tag="h1", name="h1_t"
                        )[:]
                    h1pre = h1pre_bufs[quad]
                    a_in = (
                        A2dup[:, 128 * sb2 : 128 * sb2 + 64 * nsb]
                        .rearrange("p (s i two) -> p s i two", s=nsb, two=2)
                        .unsqueeze(3)
                        .broadcast_to([128, nsb, 32, 16, 2])
                    )
                    b_in = (
                        B2s[:, 64 * sb2 : 64 * sb2 + 32 * nsb]
                        .rearrange("p (s jh jl) -> p s jh jl", s=nsb, jl=2)
                        .unsqueeze(2)
                        .broadcast_to([128, nsb, 32, 16, 2])
                    )
                    h1pre_v = h1pre[
                        :, 2 * COLS * h4 : 2 * COLS * h4 + nsb * COLS
                    ].rearrange("p (s i jh jl) -> p s i jh jl", s=nsb, i=32, jl=2)
                    (eng or nc.vector).tensor_add(h1pre_v, a_in, b_in)

                def emit_s1_relu(sb2, nsb=2):
                    quad, h4 = divmod(sb2, 2)
                    cs = slice(2 * COLS * h4, 2 * COLS * h4 + nsb * COLS)
                    nc.vector.tensor_scalar_max(
                        h1_bufs[quad][:, cs], h1pre_bufs[quad][:, cs], 0.0
                    )

                def emit_s1_single(sb):
                    # 1-sb S1 (first and last two sbs): halves the latency from
                    # "h1 ready" to the adjoining L2->evict->L3->sigmoid chain.
                    quad, k4 = divmod(sb, 4)
                    if quad not in h1pre_bufs:
                        h1pre_bufs[quad] = h1pre_pool.tile(
                            [128, 4 * COLS], BF16, tag="h1pre", name="h1pre_t"
                        )[:]
                        h1_bufs[quad] = h1_pool.tile(
                            [128, 4 * COLS], BF16, tag="h1", name="h1_t"
                        )[:]
                    h1pre = h1pre_bufs[quad]
                    a_in = (
                        A2dup[:, 64 * sb : 64 * sb + 64]
                        .rearrange("p (i two) -> p i two", two=2)
                        .unsqueeze(2)
                        .broadcast_to([128, 32, 16, 2])
                    )
                    b_in = (
                        B2s[:, 32 * sb : 32 * sb + 32]
                        .rearrange("p (jh jl) -> p jh jl", jl=2)
                        .unsqueeze(1)
                        .broadcast_to([128, 32, 16, 2])
                    )
                    cs = slice(COLS * k4, COLS * k4 + COLS)
                    h1pre_v = h1pre[:, cs].rearrange(
                        "p (i jh jl) -> p i jh jl", i=32, jl=2
                    )
                    nc.vector.tensor_add(h1pre_v, a_in, b_in)
                    nc.vector.tensor_scalar_max(h1_bufs[quad][:, cs], h1pre[:, cs], 0.0)

                def emit_l2_s2(sb, hk):
                    # K=128 M=128 block-diagonal W2: one matmul per 512 cols.
                    psum2 = l2_pool.tile([128, COLS], FP32, tag="l2", name="psum2_t")[:]
                    nc.tensor.matmul(psum2[:, 0:512], W2blk, hk[:, 0:512])
                    nc.tensor.matmul(psum2[:, 512:1024], W2blk, hk[:, 512:1024])
                    h2 = h2_pool.tile([128, COLS], BF16, tag="h2", name="h2_t")[:]
                    if sb in H2EV_DVE:
                        nc.vector.tensor_scalar(h2, psum2, bias2, 0.0, add, max_)
                    else:
                        nc.scalar.activation(h2, psum2, AF.Relu, bias=bias2)
                    h2_tiles[sb] = h2

                def emit_l3(sb):
                    # K=128 M=32: W3v packs W3 for both l-block parities into
                    # rows 0:8 / 16:24 of a 32-wide M block; tile_position col
                    # offsets place the pair's four (sb,chunk) units in one
                    # [128, 512] psum bank.
                    pair, k = divmod(sb, 2)
                    if k == 0:
                        psum3_tiles[pair] = l3_pool.tile(
                            [128, 512], FP32, tag="l3", name="psum3_t"
                        )[:]
                    psum3 = psum3_tiles[pair]
                    h2 = h2_tiles.pop(sb)
                    for ch in (0, 1):
                        u = 2 * k + ch
                        nc.tensor.matmul(
                            psum3[32 * u : 32 * u + 32, :],
                            W3v,
                            h2[:, 512 * ch : 512 * ch + 512],
                            tile_position=(0, 32 * u),
                        )

                def emit_sigmoid_dma(pair):
                    sig = sig_pool.tile([128, 512], BF16, tag="sig", name="sig_t")[:]
                    psum3 = psum3_tiles.pop(pair)
                    nc.scalar.activation(sig, psum3, AF.Sigmoid, bias=bias3)
                    if pair == NPAIR - 1:
                        # Split the last transfer so the drain tail halves.
                        nc.sync.dma_start(out=d_out[pair, 0:64], in_=sig[0:64])
                        nc.gpsimd.dma_start(out=d_out[pair, 64:128], in_=sig[64:128])
                    else:
                        eng = nc.sync if pair % 2 == 0 else nc.gpsimd
                        eng.dma_start(out=d_out[pair], in_=sig)

                # Software pipeline, deliberately deep so no strict-FIFO queue
                # ever stalls at its head: S1 runs two quads (8 sbs) ahead of
                # L2, in per-2sb chunks; L3 lags L2 by two sbs (its h2 is long
                # since evicted); the sigmoid lags its pair's last L3 too.
                # S1 emission plan: 2-sb chunks up front (fast pipeline fill),
                # 4-sb (full-quad) chunks mid-stream, per-sb ops at the tail.
                s1_plan = {2: 4, 6: 6, 10: 8, 14: 10, 18: 12}
                emit_s1_single(0)
                emit_s1_single(1)
                for sb2 in range(1, 4):
                    emit_s1_tt(sb2)
                    emit_s1_relu(sb2)
                for sb in range(NSB):
                    quad, k4 = divmod(sb, 4)
                    if sb in s1_plan:
                        emit_s1_tt(s1_plan[sb], nsb=4)
                        emit_s1_relu(s1_plan[sb], nsb=4)
                    elif sb == 22:
                        emit_s1_tt(14)
                        emit_s1_relu(14)
                    elif sb == 24:
                        emit_s1_single(30)
                        emit_s1_single(31)
                    emit_l2_s2(sb, h1_bufs[quad][:, COLS * k4 : COLS * k4 + COLS])
                    if sb >= 2:
                        emit_l3(sb - 2)
                    if sb % 2 == 1 and sb >= 3:  # pair (sb-3)//2 L3'd at sb-1
                        emit_sigmoid_dma((sb - 3) // 2)
                emit_l3(NSB - 2)
                emit_l3(NSB - 1)
                emit_sigmoid_dma(NPAIR - 1)

    nc.compile()
    input_names = ["statesQ", "Wl1", "W2blk", "W3v", "biases"]
    return nc, input_names


def get_program():
    global _PROGRAM
    if _PROGRAM is None:
        _PROGRAM = _build_program()
    return _PROGRAM


def make_inputs(states, W1, b1, W2, b2, W3, b3):
    """Host-side prep: per-core statesQ + shared packed weights/biases."""
    states = np.asarray(states, np.float32)
    W1 = np.asarray(W1, np.float32)
    W2 = np.asarray(W2, np.float32)
    W3 = np.asarray(W3, np.float32)
    b1 = np.asarray(b1, np.float32)
    b2 = np.asarray(b2, np.float32)
    b3 = np.asarray(b3, np.float32)

    # Wl1 [32, 256]: cols 0:128 = K=32 parity-blockdiag lhsT for A2
    # (rows 0:16 -> outs 0:64 even, rows 16:32 -> outs 64:128 odd),
    # cols 128:256 = same structure for B2 (second half of W1).
    Wl1 = np.zeros((32, 256), NP_BF16)
    Wl1[0:16, 0:64] = W1[:D].astype(NP_BF16)
    Wl1[16:32, 64:128] = W1[:D].astype(NP_BF16)
    Wl1[0:16, 128:192] = W1[D:].astype(NP_BF16)
    Wl1[16:32, 192:256] = W1[D:].astype(NP_BF16)

    W2blk = np.zeros((128, 128), NP_BF16)
    W2blk[0:64, 0:64] = W2.astype(NP_BF16)
    W2blk[64:128, 64:128] = W2.astype(NP_BF16)

    # W3v: K=128 M=32 weight for L3 — even-parity h2 (parts 0:64) feeds out
    # rows 0:8, odd-parity (64:128) feeds rows 16:24.
    W3v = np.zeros((128, 32), NP_BF16)
    W3v[0:64, 0:8] = W3.astype(NP_BF16)
    W3v[64:128, 16:24] = W3.astype(NP_BF16)

    biases = np.zeros((128, 3), np.float32)
    biases[:, 0] = np.tile(b1, 2)
    biases[:, 1] = np.tile(b2, 2)
    b3pat = np.zeros(32, np.float32)
    b3pat[0:8] = b3
    b3pat[16:24] = b3
    biases[:, 2] = np.tile(b3pat, 4)

    shared = {"Wl1": Wl1, "W2blk": W2blk, "W3v": W3v, "biases": biases}

    in_maps = []
    for c in range(NCORES):
        # statesT[d, 32*l + i] = states[c, l, i, d]
        statesT = states[c].reshape(L * N, D).T.astype(NP_BF16)
        sQ = np.zeros((32, 2048), NP_BF16)
        sQ[0:16] = statesT
        sQ[16:32, : 2048 - 32] = statesT[:, 32:]
        in_maps.append({"statesQ": sQ, **shared})
    return in_maps


def decode_output(raw):
    """Invert the device output layout -> [L, N, N, F] for one core.

    raw: [NPAIR, 128, 512] bf16. Partitions: u = 2*s + ch in blocks of 32
    (s = sb within pair, ch = column chunk), rows 0:8 = even l-block
    features, 16:24 = odd. col c: pair-col = 512*ch + c = 32*i + j.
    l = 4*pair + 2*s + parity.
    """
    raw = raw.astype(np.float32)
    ov = raw.reshape(NPAIR, 2, 2, 32, 512)       # [pair, s, ch, row, c]
    ev = ov[:, :, :, 0:8, :]                     # even parity
    od = ov[:, :, :, 16:24, :]                   # odd parity
    st = np.stack([ev, od], axis=3)              # [pair, s, ch, par, f, c]
    st = st.transpose(0, 1, 3, 2, 5, 4)          # [pair, s, par, ch, c, f]
    return np.ascontiguousarray(st.reshape(L, N * N, F)).reshape(L, N, N, F)


def _ensure_ntff_hook():
    """Best-effort shim for the missing antenv.axon_hooks module so
    run_bass_kernel_spmd(trace=True) can capture NTFF profiles under axon."""
    import types

    try:
        from antenv.axon_hooks import get_axon_ntff_profile_hook  # noqa: F401
        return
    except ImportError:
        pass
    try:
        if "/root/.axon_site" not in sys.path:
            sys.path.insert(0, "/root/.axon_site")
        from trn_agent_boot.trn_boot import _ntff_profile_via_ctypes

        hook = _ntff_profile_via_ctypes("/opt/axon/libaxon_pjrt.so")
        import antenv

        mod = types.ModuleType("antenv.axon_hooks")
        mod._hook = hook
        mod.set_axon_ntff_profile_hook = lambda h: setattr(mod, "_hook", h)
        mod.get_axon_ntff_profile_hook = lambda: mod._hook
        sys.modules["antenv.axon_hooks"] = mod
        antenv.axon_hooks = mod
    except Exception as e:  # tracing is optional; never break the run
        print(f"ntff hook shim failed: {e}", file=sys.stderr)


def kernel(states, W1, b1, W2, b2, W3, b3):
    global LAST_RESULT
    nc, _ = get_program()
    if os.environ.get("KERNEL_TRACE"):
        _ensure_ntff_hook()
    in_maps = make_inputs(states, W1, b1, W2, b2, W3, b3)
    res = run_bass_kernel_spmd(
        nc,
        in_maps,
        core_ids=list(range(NCORES)),
        trace=bool(os.environ.get("KERNEL_TRACE")),
    )
    LAST_RESULT = res
    out = np.empty((B, L, N, N, F), np.float32)
    for c in range(NCORES):
        out[c] = decode_output(res.results[c]["out"])
    return out
